# revision 11
# baseline (speedup 1.0000x reference)
"""Bass/Trainium2 SPMD kernel for nn_AttnDecoderRNN (B=512, H=1024, L=15, V=32000).

Strategy (8 NeuronCores):
- Phase 1 (embedding gather, additive attention, combine+ReLU, GRU cell):
  data-parallel over batch, 64 rows per core. Weights replicated (bf16).
- h_new exchanged via AllGather (bf16, transposed layout).
- Phase 2 (out_w matmul + log_softmax): tensor-parallel over vocab,
  4000 columns per core; log-softmax denominator combined with a tiny
  AllReduce of per-core sum(exp(logits)); final output written as
  log(exp(logits) * 1/global_sum) on the scalar engine.

Numerics: matmuls run in bf16 with fp32 PSUM accumulation; softmax, GRU
gate nonlinearities, exp/log-softmax all in fp32. Additive-attention
scores drop the embedding-dot term: it is constant across L for a given
batch row, so softmax over L is invariant to it (same for attn1_b).
"""

import contextlib

import numpy as np
import ml_dtypes

import concourse.bass as bass
import concourse.tile as tile
from concourse import bacc, mybir
from concourse.bass_utils import run_bass_kernel_spmd
from concourse.masks import make_identity

BF16 = ml_dtypes.bfloat16

N_CORES = 8
B = 512
H = 1024
L = 15
V = 32000
BC = B // N_CORES        # 64 batch rows per core
VC = V // N_CORES        # 4000 vocab cols per core
P = 128
KH = H // P              # 8 k-chunks over H
KC = 2 * H // P          # 16 k-chunks over 2H
NV = 8                   # vocab tiles per core
VT = VC // NV            # 500 cols per vocab tile
MB = B // P              # 4 batch chunks of 128 in phase 2

F32 = mybir.dt.float32
BF = mybir.dt.bfloat16
I32 = mybir.dt.int32

_CACHE = {}


def _phase1(nc, tc, ctx, d, consts, dram, wstream):
    """Gather + attention + combine + GRU for this core's 64 batch rows.

    Returns the DRAM tile holding h_new^T (AllGather input).
    """
    ident64, ident128, ones_row = consts
    p1 = ctx.enter_context(tc.tile_pool(name="p1", bufs=1))
    gates = ctx.enter_context(tc.tile_pool(name="gates", bufs=6))
    psum = ctx.enter_context(tc.tile_pool(name="psum1", bufs=2, space="PSUM"))
    psum_g = ctx.enter_context(tc.tile_pool(name="psum_g", bufs=3, space="PSUM"))
    psum_acc = ctx.enter_context(
        tc.tile_pool(name="psum_acc", bufs=1, space="PSUM"))

    ids_sb = p1.tile([BC, 1], I32)
    nc.sync.dma_start(ids_sb[:], d["ids"][:])
    emb_sb = p1.tile([BC, H], BF)
    nc.gpsimd.indirect_dma_start(
        out=emb_sb[:], out_offset=None,
        in_=d["emb"][:],
        in_offset=bass.IndirectOffsetOnAxis(ap=ids_sb[:, :1], axis=0),
    )
    encT_sb = p1.tile([P, KH, BC, L], BF)
    nc.sync.dma_start(
        encT_sb[:], d["encT"].ap().rearrange("(k p) b l -> p k b l", p=P))
    enc_sb = p1.tile([BC, L, H], BF)
    nc.sync.dma_start(enc_sb[:], d["enc"].ap().rearrange("l b h -> b l h"))
    wb_sb = p1.tile([P, KH], BF)
    nc.sync.dma_start(wb_sb[:], d["wb"].ap().rearrange("(k p) -> p k", p=P))
    bcomb_sb = p1.tile([P, KH], F32)
    nc.sync.dma_start(bcomb_sb[:], d["bcomb"].ap().rearrange("(m p) -> p m", p=P))
    bih_sb = p1.tile([1, 3 * H], BF)
    nc.sync.dma_start(bih_sb[:], d["bih"][:])
    bhh_sb = p1.tile([1, 3 * H], BF)
    nc.sync.dma_start(bhh_sb[:], d["bhh"][:])
    h_sb = p1.tile([BC, H], F32)
    nc.sync.dma_start(h_sb[:], d["h0"][:])
    hT_sb = p1.tile([P, KH, BC], BF)
    nc.sync.dma_start(hT_sb[:], d["h0T"].ap().rearrange("(k p) b -> p k b", p=P))

    # attention scores: s[b,l] = sum_h enc[l,b,h]*wb[h]
    psum_sc = psum_acc.tile([BC, L], F32, space="PSUM", name="psum_sc")
    for l in range(L):
        for k in range(KH):
            nc.tensor.matmul(
                psum_sc[:, l:l + 1],
                lhsT=encT_sb[:, k, :, l],
                rhs=wb_sb[:, k:k + 1],
                start=(k == 0), stop=(k == KH - 1),
            )
    # softmax over L (fp32)
    sc_sb = p1.tile([BC, L], F32)
    nc.vector.tensor_copy(sc_sb[:], psum_sc[:])
    mx = p1.tile([BC, 1], F32)
    nc.vector.reduce_max(mx[:], sc_sb[:], axis=mybir.AxisListType.X)
    sc2 = p1.tile([BC, L], F32)
    nc.vector.tensor_scalar(
        out=sc2[:], in0=sc_sb[:], scalar1=mx[:, :1], scalar2=None,
        op0=mybir.AluOpType.subtract)
    ex = p1.tile([BC, L], F32)
    sm = p1.tile([BC, 1], F32)
    nc.scalar.activation(ex[:], sc2[:], mybir.ActivationFunctionType.Exp,
                         accum_out=sm[:, :1])
    rs = p1.tile([BC, 1], F32)
    nc.vector.reciprocal(rs[:], sm[:])
    attn = p1.tile([BC, L], F32)
    nc.vector.tensor_scalar(
        out=attn[:], in0=ex[:], scalar1=rs[:, :1], scalar2=None,
        op0=mybir.AluOpType.mult)
    nc.sync.dma_start(d["o_attn"][:], attn[:])
    attn_b = p1.tile([BC, L], BF)
    nc.vector.tensor_copy(attn_b[:], attn[:])

    # ctx^T via PE: ctxT[h,b] = sum_l enc[l,b,h]*attn[l,b]
    adiag = p1.tile([BC, L, BC], BF)
    for l in range(L):
        nc.vector.tensor_tensor(
            out=adiag[:, l, :], in0=ident64[:],
            in1=attn_b[:, l:l + 1].to_broadcast([BC, BC]),
            op=mybir.AluOpType.mult)
    if "o_adiag" in d:
        nc.sync.dma_start(d["o_adiag"][:],
                          adiag[:].rearrange("b l c -> b (l c)"))
        nc.sync.dma_start(d["o_encrt"][:],
                          enc_sb[:].rearrange("b l h -> b (l h)"))
    psum_ctxT = psum_acc.tile([P, KH * BC], F32, space="PSUM", name="psum_ctxT")
    for k in range(KH):
        for l in range(L):
            nc.tensor.matmul(
                psum_ctxT[:, k * BC:(k + 1) * BC],
                lhsT=enc_sb[:, l, k * P:(k + 1) * P],
                rhs=adiag[:, l, :],
                start=(l == 0), stop=(l == L - 1),
            )

    # cat^T = [emb^T ; ctx^T]  (bf16, [128, 16, 64])
    catT = p1.tile([P, KC, BC], BF)
    for k in range(KH):
        pt = psum.tile([P, BC], BF, space="PSUM", name="pt_embT", tag="pt")
        nc.tensor.transpose(pt[:], emb_sb[:, k * P:(k + 1) * P], ident64[:])
        nc.scalar.copy(catT[:, k, :], pt[:])
    for k in range(KH):
        nc.scalar.copy(catT[:, KH + k, :], psum_ctxT[:, k * BC:(k + 1) * BC])

    # combine + ReLU -> x^T (bf16 [128, 8, 64])
    xT = p1.tile([P, KH, BC], BF)
    for m in range(KH):
        wcomb_sb = wstream.tile([P, KC, P], BF, name="wcomb_t", tag="w")
        nc.sync.dma_start(
            wcomb_sb[:],
            d["wcomb"].ap()[:, m * P:(m + 1) * P]
            .rearrange("(k p) q -> p k q", p=P))
        px = psum.tile([P, BC], F32, space="PSUM", name="px", tag="pt")
        for k in range(KC):
            nc.tensor.matmul(px[:], lhsT=wcomb_sb[:, k, :], rhs=catT[:, k, :],
                             start=(k == 0), stop=(k == KC - 1))
        nc.scalar.activation(xT[:, m, :], px[:],
                             mybir.ActivationFunctionType.Relu,
                             bias=bcomb_sb[:, m:m + 1])

    if "o_xT" in d:
        xT_f = p1.tile([P, KH, BC], F32, name="xT_f")
        nc.vector.tensor_copy(xT_f[:], xT[:])
        nc.sync.dma_start(
            d["o_xT"].ap().rearrange("(k p) b -> p k b", p=P), xT_f[:])
        catT_f = p1.tile([P, KC, BC], F32, name="catT_f")
        nc.vector.tensor_copy(catT_f[:], catT[:])
        nc.sync.dma_start(
            d["o_catT"].ap().rearrange("(k p) b -> p k b", p=P), catT_f[:])

    # GRU: gi = x@w_ih^T + b_ih, gh = h@w_hh^T + b_hh  (b-layout [64, 3072])
    gi_sb = p1.tile([BC, 3 * H], F32)
    gh_sb = p1.tile([BC, 3 * H], F32)
    NG = 3 * H // 512
    for n in range(NG):
        sl = slice(n * 512, (n + 1) * 512)
        wih_t = wstream.tile([P, KH, 512], BF, name="wih_t", tag="w")
        nc.sync.dma_start(
            wih_t[:], d["wih"].ap()[:, sl].rearrange("(k p) q -> p k q", p=P))
        pg = psum_g.tile([BC, 512], F32, space="PSUM", name="pg", tag="pgh")
        for k in range(KH):
            nc.tensor.matmul(pg[:], lhsT=xT[:, k, :], rhs=wih_t[:, k, :],
                             start=(k == 0), stop=False)
        nc.tensor.matmul(pg[:], lhsT=ones_row[:1, :BC], rhs=bih_sb[:1, sl],
                         start=False, stop=True)
        nc.scalar.copy(gi_sb[:, sl], pg[:])

        whh_t = wstream.tile([P, KH, 512], BF, name="whh_t", tag="w")
        nc.sync.dma_start(
            whh_t[:], d["whh"].ap()[:, sl].rearrange("(k p) q -> p k q", p=P))
        ph = psum_g.tile([BC, 512], F32, space="PSUM", name="ph", tag="pgh")
        for k in range(KH):
            nc.tensor.matmul(ph[:], lhsT=hT_sb[:, k, :], rhs=whh_t[:, k, :],
                             start=(k == 0), stop=False)
        nc.tensor.matmul(ph[:], lhsT=ones_row[:1, :BC], rhs=bhh_sb[:1, sl],
                         start=False, stop=True)
        nc.scalar.copy(gh_sb[:, sl], ph[:])

    if "o_gi" in d:
        nc.sync.dma_start(d["o_gi"][:], gi_sb[:])
        nc.sync.dma_start(d["o_gh"][:], gh_sb[:])

    # gates (fp32, b-layout)
    s1, s2, s3 = slice(0, H), slice(H, 2 * H), slice(2 * H, 3 * H)

    def gt(name):
        return gates.tile([BC, H], F32, name=name, tag="g")

    rg_t = gt("rg_t")
    nc.vector.tensor_add(rg_t[:], gi_sb[:, s1], gh_sb[:, s1])
    r_sb = gt("r_sb")
    nc.scalar.activation(r_sb[:], rg_t[:], mybir.ActivationFunctionType.Sigmoid)
    zg_t = gt("zg_t")
    nc.vector.tensor_add(zg_t[:], gi_sb[:, s2], gh_sb[:, s2])
    z_sb = gt("z_sb")
    nc.scalar.activation(z_sb[:], zg_t[:], mybir.ActivationFunctionType.Sigmoid)
    rn_t = gt("rn_t")
    nc.vector.tensor_mul(rn_t[:], r_sb[:], gh_sb[:, s3])
    nn_t = gt("nn_t")
    nc.vector.tensor_add(nn_t[:], gi_sb[:, s3], rn_t[:])
    n_sb = gt("n_sb")
    nc.scalar.activation(n_sb[:], nn_t[:], mybir.ActivationFunctionType.Tanh)
    zh_t = gt("zh_t")
    nc.vector.tensor_mul(zh_t[:], z_sb[:], h_sb[:])
    zn_t = gt("zn_t")
    nc.vector.tensor_mul(zn_t[:], z_sb[:], n_sb[:])
    nm_t = gt("nm_t")
    nc.vector.tensor_sub(nm_t[:], n_sb[:], zn_t[:])
    hnew = gt("hnew")
    nc.vector.tensor_add(hnew[:], nm_t[:], zh_t[:])
    nc.sync.dma_start(d["o_h"][:], hnew[:])

    # h_new^T (bf16) -> DRAM for AllGather
    hT_loc = p1.tile([P, KH, BC], BF)
    for k in range(KH):
        pt2 = psum.tile([P, BC], F32, space="PSUM", name="pt_hT", tag="pt")
        nc.tensor.transpose(pt2[:], hnew[:, k * P:(k + 1) * P],
                            ident128[:BC, :BC])
        nc.scalar.copy(hT_loc[:, k, :], pt2[:])
    ag_in = dram.tile([H, BC], BF)
    nc.sync.dma_start(ag_in.rearrange("(k p) b -> p k b", p=P), hT_loc[:])
    return ag_in


def build_nc(debug_taps=False):
    nc = bacc.Bacc("TRN2", target_bir_lowering=False, debug=False,
                   num_devices=N_CORES)

    d = {}
    d["ids"] = nc.dram_tensor("ids", [BC, 1], I32, kind="ExternalInput")
    d["emb"] = nc.dram_tensor("emb", [V, H], BF, kind="ExternalInput")
    d["enc"] = nc.dram_tensor("enc", [L, BC, H], BF, kind="ExternalInput")
    d["encT"] = nc.dram_tensor("encT", [H, BC, L], BF, kind="ExternalInput")
    d["wb"] = nc.dram_tensor("wb", [H], BF, kind="ExternalInput")
    d["wcomb"] = nc.dram_tensor("wcomb", [2 * H, H], BF, kind="ExternalInput")
    d["bcomb"] = nc.dram_tensor("bcomb", [H], F32, kind="ExternalInput")
    d["wih"] = nc.dram_tensor("wih", [H, 3 * H], BF, kind="ExternalInput")
    d["whh"] = nc.dram_tensor("whh", [H, 3 * H], BF, kind="ExternalInput")
    d["bih"] = nc.dram_tensor("bih", [1, 3 * H], BF, kind="ExternalInput")
    d["bhh"] = nc.dram_tensor("bhh", [1, 3 * H], BF, kind="ExternalInput")
    d["h0"] = nc.dram_tensor("h0", [BC, H], F32, kind="ExternalInput")
    d["h0T"] = nc.dram_tensor("h0T", [H, BC], BF, kind="ExternalInput")
    d["wout"] = nc.dram_tensor("wout", [H, VC], BF, kind="ExternalInput")
    d["outb"] = nc.dram_tensor("outb", [1, VC], BF, kind="ExternalInput")
    d["o_logp"] = nc.dram_tensor("out_logp", [B, VC], F32, kind="ExternalOutput")
    if debug_taps:
        d["o_xT"] = nc.dram_tensor("out_xT", [H, BC], F32, kind="ExternalOutput")
        d["o_catT"] = nc.dram_tensor("out_catT", [2 * H, BC], F32,
                                     kind="ExternalOutput")
        d["o_gi"] = nc.dram_tensor("out_gi", [BC, 3 * H], F32,
                                   kind="ExternalOutput")
        d["o_gh"] = nc.dram_tensor("out_gh", [BC, 3 * H], F32,
                                   kind="ExternalOutput")
        d["o_adiag"] = nc.dram_tensor("out_adiag", [BC, L * BC], BF,
                                      kind="ExternalOutput")
        d["o_encrt"] = nc.dram_tensor("out_encrt", [BC, L * H], BF,
                                      kind="ExternalOutput")
    d["o_h"] = nc.dram_tensor("out_h", [BC, H], F32, kind="ExternalOutput")
    d["o_attn"] = nc.dram_tensor("out_attn", [BC, L], F32, kind="ExternalOutput")

    RG = [list(range(N_CORES))]

    with tile.TileContext(nc) as tc:
        with contextlib.ExitStack() as ctx:
            dram = ctx.enter_context(tc.tile_pool(name="dram", bufs=1, space="DRAM"))
            const = ctx.enter_context(tc.tile_pool(name="const", bufs=1))
            persist = ctx.enter_context(tc.tile_pool(name="persist", bufs=1))
            work = ctx.enter_context(tc.tile_pool(name="work", bufs=2))
            wstream = ctx.enter_context(tc.tile_pool(name="wstream", bufs=3))

            ident64 = const.tile([64, 64], BF)
            make_identity(nc, ident64)
            ident128 = const.tile([P, P], F32)
            make_identity(nc, ident128)
            ones_row = const.tile([1, P], BF)
            nc.vector.memset(ones_row[:], 1.0)

            outb_sb = persist.tile([1, VC], BF)
            nc.sync.dma_start(outb_sb[:], d["outb"][:])

            with contextlib.ExitStack() as ctx1:
                ag_in = _phase1(nc, tc, ctx1, d, (ident64, ident128, ones_row),
                                dram, wstream)

            # ---------- AllGather h_new^T across cores ----------
            ag_out = dram.tile([N_CORES * H, BC], BF, addr_space="Shared")
            nc.gpsimd.collective_compute(
                "AllGather", mybir.AluOpType.bypass,
                replica_groups=RG, ins=[ag_in.opt()], outs=[ag_out.opt()],
            )

            p2 = ctx.enter_context(tc.tile_pool(name="p2", bufs=1))
            psum2 = ctx.enter_context(
                tc.tile_pool(name="psum2", bufs=4, space="PSUM"))
            hT_all = p2.tile([P, KH, N_CORES, BC], BF)
            ag_view = ag_out.rearrange("(r k p) b -> p k r b", r=N_CORES, p=P)
            for k in range(KH):
                nc.sync.dma_start(hT_all[:, k, :, :], ag_view[:, k, :, :])

            # ---------- phase 2: logits + exp + partial sums ----------
            exp_sb = p2.tile([P, MB, VC], F32)
            sums = p2.tile([P, MB, NV], F32)
            for m in range(MB):
                for n in range(NV):
                    wt = wstream.tile([P, KH, VT], BF, name="wout_t", tag="w")
                    nc.sync.dma_start(
                        wt[:],
                        d["wout"].ap()[:, n * VT:(n + 1) * VT]
                        .rearrange("(k p) v -> p k v", p=P))
                    pl = psum2.tile([P, VT], F32, space="PSUM", name="pl")
                    for k in range(KH):
                        nc.tensor.matmul(
                            pl[:], lhsT=hT_all[:, k, 2 * m:2 * m + 2, :],
                            rhs=wt[:, k, :], start=(k == 0), stop=False)
                    nc.tensor.matmul(pl[:], lhsT=ones_row[:1, :],
                                     rhs=outb_sb[:1, n * VT:(n + 1) * VT],
                                     start=False, stop=True)
                    nc.scalar.activation(
                        exp_sb[:, m, n * VT:(n + 1) * VT], pl[:],
                        mybir.ActivationFunctionType.Exp,
                        accum_out=sums[:, m, n:n + 1])

            # ---------- global log-sum-exp via AllReduce ----------
            lsum = p2.tile([P, MB], F32)
            nc.vector.reduce_sum(lsum[:], sums[:], axis=mybir.AxisListType.X)
            ar_in = dram.tile([B, 1], F32)
            nc.sync.dma_start(ar_in.rearrange("(m p) o -> p (m o)", p=P), lsum[:])
            ar_out = dram.tile([B, 1], F32, addr_space="Shared")
            nc.gpsimd.collective_compute(
                "AllReduce", mybir.AluOpType.add,
                replica_groups=RG, ins=[ar_in.opt()], outs=[ar_out.opt()],
            )
            gsum = p2.tile([P, MB], F32)
            nc.sync.dma_start(gsum[:], ar_out.rearrange("(m p) o -> p (m o)", p=P))
            ginv = p2.tile([P, MB], F32)
            nc.vector.reciprocal(ginv[:], gsum[:])

            # ---------- final: out = ln(exp * 1/gsum) ----------
            for m in range(MB):
                for n in range(NV):
                    ot = work.tile([P, VT], F32, name="ot")
                    nc.scalar.activation(
                        ot[:], exp_sb[:, m, n * VT:(n + 1) * VT],
                        mybir.ActivationFunctionType.Ln,
                        scale=ginv[:, m:m + 1])
                    nc.sync.dma_start(
                        d["o_logp"].ap()[m * P:(m + 1) * P,
                                         n * VT:(n + 1) * VT],
                        ot[:])

    nc.compile()
    return nc


def _prep_in_maps(input_ids, hidden, encoder_outputs, embedding, attn1_w,
                  combine_w, combine_b, w_ih, w_hh, b_ih, b_hh, out_w, out_b):
    emb_b = np.ascontiguousarray(embedding.astype(BF16))
    wcomb = np.ascontiguousarray(combine_w.T.astype(BF16))
    wih = np.ascontiguousarray(w_ih.T.astype(BF16))
    whh = np.ascontiguousarray(w_hh.T.astype(BF16))
    bih = np.ascontiguousarray(b_ih.reshape(1, -1).astype(BF16))
    bhh = np.ascontiguousarray(b_hh.reshape(1, -1).astype(BF16))
    wb = np.ascontiguousarray(attn1_w[0, H:].astype(BF16))
    bcomb = np.ascontiguousarray(combine_b.astype(np.float32))
    in_maps = []
    for c in range(N_CORES):
        sl = slice(c * BC, (c + 1) * BC)
        vsl = slice(c * VC, (c + 1) * VC)
        enc_c = encoder_outputs[:, sl, :]
        h_c = hidden[0, sl, :]
        in_maps.append({
            "ids": np.ascontiguousarray(
                input_ids[sl].reshape(BC, 1).astype(np.int32)),
            "emb": emb_b,
            "enc": np.ascontiguousarray(enc_c.astype(BF16)),
            "encT": np.ascontiguousarray(enc_c.transpose(2, 1, 0).astype(BF16)),
            "wb": wb,
            "wcomb": wcomb,
            "bcomb": bcomb,
            "wih": wih,
            "whh": whh,
            "bih": bih,
            "bhh": bhh,
            "h0": np.ascontiguousarray(h_c.astype(np.float32)),
            "h0T": np.ascontiguousarray(h_c.T.astype(BF16)),
            "wout": np.ascontiguousarray(out_w[vsl, :].T.astype(BF16)),
            "outb": np.ascontiguousarray(out_b[vsl].reshape(1, -1).astype(BF16)),
        })
    return in_maps


def kernel(input_ids, hidden, encoder_outputs, max_length, embedding,
           attn1_w, attn1_b, combine_w, combine_b,
           w_ih, w_hh, b_ih, b_hh, out_w, out_b, _trace=False):
    input_ids = np.asarray(input_ids)
    hidden = np.asarray(hidden, dtype=np.float32)
    encoder_outputs = np.asarray(encoder_outputs, dtype=np.float32)
    embedding = np.asarray(embedding, dtype=np.float32)
    attn1_w = np.asarray(attn1_w, dtype=np.float32)
    combine_w = np.asarray(combine_w, dtype=np.float32)
    combine_b = np.asarray(combine_b, dtype=np.float32)
    w_ih = np.asarray(w_ih, dtype=np.float32)
    w_hh = np.asarray(w_hh, dtype=np.float32)
    b_ih = np.asarray(b_ih, dtype=np.float32)
    b_hh = np.asarray(b_hh, dtype=np.float32)
    out_w = np.asarray(out_w, dtype=np.float32)
    out_b = np.asarray(out_b, dtype=np.float32)
    assert int(max_length) == L

    if "nc" not in _CACHE:
        _CACHE["nc"] = build_nc()
    nc = _CACHE["nc"]

    in_maps = _prep_in_maps(input_ids, hidden, encoder_outputs, embedding,
                            attn1_w, combine_w, combine_b, w_ih, w_hh,
                            b_ih, b_hh, out_w, out_b)
    res = run_bass_kernel_spmd(nc, in_maps, list(range(N_CORES)),
                               trace=_trace)
    if _trace:
        print(f"HW exec time: {res.exec_time_ns} ns")

    output = np.concatenate(
        [res.results[c]["out_logp"] for c in range(N_CORES)], axis=1)
    h_new = np.concatenate(
        [res.results[c]["out_h"] for c in range(N_CORES)], axis=0)[None]
    attn_w = np.concatenate(
        [res.results[c]["out_attn"] for c in range(N_CORES)], axis=0)
    attn_w = np.ascontiguousarray(attn_w.T)[:, :, None]
    return (output.astype(np.float32), h_new.astype(np.float32),
            attn_w.astype(np.float32))


# revision 17
# speedup vs baseline: 1.1518x; 1.1518x over previous
"""Bass/Trainium2 SPMD kernel for nn_AttnDecoderRNN (B=512, H=1024, L=15, V=32000).

Strategy (8 NeuronCores):
- Phase 1 (embedding gather, additive attention, combine+ReLU, GRU cell):
  data-parallel over batch, 64 rows per core. Weights replicated (bf16).
- h_new exchanged via AllGather (bf16, transposed layout).
- Phase 2 (out_w matmul + log_softmax): tensor-parallel over vocab,
  4000 columns per core; log-softmax denominator combined with a tiny
  AllReduce of per-core sum(exp(logits)); final output written as
  log(exp(logits) * 1/global_sum) on the scalar engine.

Numerics: matmuls run in bf16 with fp32 PSUM accumulation; softmax, GRU
gate nonlinearities, exp/log-softmax all in fp32. Additive-attention
scores drop the embedding-dot term: it is constant across L for a given
batch row, so softmax over L is invariant to it (same for attn1_b).
"""

import contextlib

import numpy as np
import ml_dtypes

import concourse.bass as bass
import concourse.tile as tile
from concourse import bacc, mybir
from concourse.bass_utils import run_bass_kernel_spmd
from concourse.masks import make_identity

BF16 = ml_dtypes.bfloat16

N_CORES = 8
B = 512
H = 1024
L = 15
V = 32000
BC = B // N_CORES        # 64 batch rows per core
VC = V // N_CORES        # 4000 vocab cols per core
P = 128
KH = H // P              # 8 k-chunks over H
KC = 2 * H // P          # 16 k-chunks over 2H
NV = 8                   # vocab tiles per core
VT = VC // NV            # 500 cols per vocab tile
MB = B // P              # 4 batch chunks of 128 in phase 2

F32 = mybir.dt.float32
BF = mybir.dt.bfloat16
I32 = mybir.dt.int32

_CACHE = {}


def _phase1(nc, tc, ctx, d, consts, dram, wstream):
    """Gather + attention + combine + GRU for this core's 64 batch rows.

    Returns the DRAM tile holding h_new^T (AllGather input).
    """
    ident64, ident128, ones_row = consts
    p1 = ctx.enter_context(tc.tile_pool(name="p1", bufs=1))
    gates = ctx.enter_context(tc.tile_pool(name="gates", bufs=6))
    psum = ctx.enter_context(tc.tile_pool(name="psum1", bufs=2, space="PSUM"))
    psum_g = ctx.enter_context(tc.tile_pool(name="psum_g", bufs=3, space="PSUM"))
    psum_acc = ctx.enter_context(
        tc.tile_pool(name="psum_acc", bufs=1, space="PSUM"))

    ids_sb = p1.tile([BC, 1], I32)
    nc.sync.dma_start(ids_sb[:], d["ids"][:])
    emb_sb = p1.tile([BC, H], BF)
    nc.gpsimd.indirect_dma_start(
        out=emb_sb[:], out_offset=None,
        in_=d["emb"][:],
        in_offset=bass.IndirectOffsetOnAxis(ap=ids_sb[:, :1], axis=0),
    )
    encT_sb = p1.tile([P, KH, BC, L], BF)
    nc.sync.dma_start(
        encT_sb[:], d["encT"].ap().rearrange("(k p) b l -> p k b l", p=P))
    enc_sb = p1.tile([BC, L, H], BF)
    nc.sync.dma_start(enc_sb[:], d["enc"].ap().rearrange("l b h -> b l h"))
    wb_sb = p1.tile([P, KH], BF)
    nc.sync.dma_start(wb_sb[:], d["wb"].ap().rearrange("(k p) -> p k", p=P))
    bcomb_sb = p1.tile([P, KH], F32)
    nc.sync.dma_start(bcomb_sb[:], d["bcomb"].ap().rearrange("(m p) -> p m", p=P))
    bih_sb = p1.tile([1, 3 * H], BF)
    nc.sync.dma_start(bih_sb[:], d["bih"][:])
    bhh_sb = p1.tile([1, 3 * H], BF)
    nc.sync.dma_start(bhh_sb[:], d["bhh"][:])
    h_sb = p1.tile([BC, H], F32)
    nc.sync.dma_start(h_sb[:], d["h0"][:])
    hT_sb = p1.tile([P, KH, BC], BF)
    nc.sync.dma_start(hT_sb[:], d["h0T"].ap().rearrange("(k p) b -> p k b", p=P))

    # attention scores: s[b,l] = sum_h enc[l,b,h]*wb[h]
    psum_sc = psum_acc.tile([BC, L], F32, space="PSUM", name="psum_sc")
    for l in range(L):
        for k in range(KH):
            nc.tensor.matmul(
                psum_sc[:, l:l + 1],
                lhsT=encT_sb[:, k, :, l],
                rhs=wb_sb[:, k:k + 1],
                start=(k == 0), stop=(k == KH - 1),
            )
    # softmax over L (fp32)
    sc_sb = p1.tile([BC, L], F32)
    nc.vector.tensor_copy(sc_sb[:], psum_sc[:])
    mx = p1.tile([BC, 1], F32)
    nc.vector.reduce_max(mx[:], sc_sb[:], axis=mybir.AxisListType.X)
    sc2 = p1.tile([BC, L], F32)
    nc.vector.tensor_scalar(
        out=sc2[:], in0=sc_sb[:], scalar1=mx[:, :1], scalar2=None,
        op0=mybir.AluOpType.subtract)
    ex = p1.tile([BC, L], F32)
    sm = p1.tile([BC, 1], F32)
    nc.scalar.activation(ex[:], sc2[:], mybir.ActivationFunctionType.Exp,
                         accum_out=sm[:, :1])
    rs = p1.tile([BC, 1], F32)
    nc.vector.reciprocal(rs[:], sm[:])
    attn = p1.tile([BC, L], F32)
    nc.vector.tensor_scalar(
        out=attn[:], in0=ex[:], scalar1=rs[:, :1], scalar2=None,
        op0=mybir.AluOpType.mult)
    nc.sync.dma_start(d["o_attn"][:], attn[:])
    attn_b = p1.tile([BC, L], BF)
    nc.vector.tensor_copy(attn_b[:], attn[:])

    # ctx^T via PE: ctxT[h,b] = sum_l enc[l,b,h]*attn[l,b]
    adiag = p1.tile([BC, L, BC], BF)
    for l in range(L):
        nc.vector.tensor_tensor(
            out=adiag[:, l, :], in0=ident64[:],
            in1=attn_b[:, l:l + 1].to_broadcast([BC, BC]),
            op=mybir.AluOpType.mult)
    if "o_adiag" in d:
        nc.sync.dma_start(d["o_adiag"][:],
                          adiag[:].rearrange("b l c -> b (l c)"))
        nc.sync.dma_start(d["o_encrt"][:],
                          enc_sb[:].rearrange("b l h -> b (l h)"))
    psum_ctxT = psum_acc.tile([P, KH * BC], F32, space="PSUM", name="psum_ctxT")
    for k in range(KH):
        for l in range(L):
            nc.tensor.matmul(
                psum_ctxT[:, k * BC:(k + 1) * BC],
                lhsT=enc_sb[:, l, k * P:(k + 1) * P],
                rhs=adiag[:, l, :],
                start=(l == 0), stop=(l == L - 1),
            )

    # cat^T = [emb^T ; ctx^T]  (bf16, [128, 16, 64])
    catT = p1.tile([P, KC, BC], BF)
    for k in range(KH):
        pt = psum.tile([P, BC], BF, space="PSUM", name="pt_embT", tag="pt")
        nc.tensor.transpose(pt[:], emb_sb[:, k * P:(k + 1) * P], ident64[:])
        nc.vector.tensor_copy(catT[:, k, :], pt[:])
    for k in range(KH):
        nc.vector.tensor_copy(catT[:, KH + k, :],
                              psum_ctxT[:, k * BC:(k + 1) * BC])

    # combine + ReLU -> x^T (bf16 [128, 8, 64])
    xT = p1.tile([P, KH, BC], BF)
    for m in range(KH):
        wcomb_sb = wstream.tile([P, KC, P], BF, name="wcomb_t", tag="w")
        nc.sync.dma_start(
            wcomb_sb[:],
            d["wcomb"].ap()[:, m * P:(m + 1) * P]
            .rearrange("(k p) q -> p k q", p=P))
        px = psum.tile([P, BC], F32, space="PSUM", name="px", tag="pt")
        for k in range(KC):
            nc.tensor.matmul(px[:], lhsT=wcomb_sb[:, k, :], rhs=catT[:, k, :],
                             start=(k == 0), stop=(k == KC - 1))
        nc.scalar.activation(xT[:, m, :], px[:],
                             mybir.ActivationFunctionType.Relu,
                             bias=bcomb_sb[:, m:m + 1])

    if "o_xT" in d:
        xT_f = p1.tile([P, KH, BC], F32, name="xT_f")
        nc.vector.tensor_copy(xT_f[:], xT[:])
        nc.sync.dma_start(
            d["o_xT"].ap().rearrange("(k p) b -> p k b", p=P), xT_f[:])
        catT_f = p1.tile([P, KC, BC], F32, name="catT_f")
        nc.vector.tensor_copy(catT_f[:], catT[:])
        nc.sync.dma_start(
            d["o_catT"].ap().rearrange("(k p) b -> p k b", p=P), catT_f[:])

    # GRU: gi = x@w_ih^T + b_ih, gh = h@w_hh^T + b_hh  (b-layout [64, 3072])
    gi_sb = p1.tile([BC, 3 * H], F32)
    gh_sb = p1.tile([BC, 3 * H], F32)
    NG = 3 * H // 512
    for n in range(NG):
        sl = slice(n * 512, (n + 1) * 512)
        wih_t = wstream.tile([P, KH, 512], BF, name="wih_t", tag="w")
        nc.sync.dma_start(
            wih_t[:], d["wih"].ap()[:, sl].rearrange("(k p) q -> p k q", p=P))
        pg = psum_g.tile([BC, 512], F32, space="PSUM", name="pg", tag="pgh")
        for k in range(KH):
            nc.tensor.matmul(pg[:], lhsT=xT[:, k, :], rhs=wih_t[:, k, :],
                             start=(k == 0), stop=False)
        nc.tensor.matmul(pg[:], lhsT=ones_row[:1, :BC], rhs=bih_sb[:1, sl],
                         start=False, stop=True)
        nc.vector.tensor_copy(gi_sb[:, sl], pg[:])

        whh_t = wstream.tile([P, KH, 512], BF, name="whh_t", tag="w")
        nc.sync.dma_start(
            whh_t[:], d["whh"].ap()[:, sl].rearrange("(k p) q -> p k q", p=P))
        ph = psum_g.tile([BC, 512], F32, space="PSUM", name="ph", tag="pgh")
        for k in range(KH):
            nc.tensor.matmul(ph[:], lhsT=hT_sb[:, k, :], rhs=whh_t[:, k, :],
                             start=(k == 0), stop=False)
        nc.tensor.matmul(ph[:], lhsT=ones_row[:1, :BC], rhs=bhh_sb[:1, sl],
                         start=False, stop=True)
        nc.vector.tensor_copy(gh_sb[:, sl], ph[:])

    if "o_gi" in d:
        nc.sync.dma_start(d["o_gi"][:], gi_sb[:])
        nc.sync.dma_start(d["o_gh"][:], gh_sb[:])

    # gates (fp32, b-layout)
    s1, s2, s3 = slice(0, H), slice(H, 2 * H), slice(2 * H, 3 * H)

    def gt(name):
        return gates.tile([BC, H], F32, name=name, tag="g")

    rg_t = gt("rg_t")
    nc.vector.tensor_add(rg_t[:], gi_sb[:, s1], gh_sb[:, s1])
    r_sb = gt("r_sb")
    nc.scalar.activation(r_sb[:], rg_t[:], mybir.ActivationFunctionType.Sigmoid)
    zg_t = gt("zg_t")
    nc.vector.tensor_add(zg_t[:], gi_sb[:, s2], gh_sb[:, s2])
    z_sb = gt("z_sb")
    nc.scalar.activation(z_sb[:], zg_t[:], mybir.ActivationFunctionType.Sigmoid)
    rn_t = gt("rn_t")
    nc.vector.tensor_mul(rn_t[:], r_sb[:], gh_sb[:, s3])
    nn_t = gt("nn_t")
    nc.vector.tensor_add(nn_t[:], gi_sb[:, s3], rn_t[:])
    n_sb = gt("n_sb")
    nc.scalar.activation(n_sb[:], nn_t[:], mybir.ActivationFunctionType.Tanh)
    zh_t = gt("zh_t")
    nc.vector.tensor_mul(zh_t[:], z_sb[:], h_sb[:])
    zn_t = gt("zn_t")
    nc.vector.tensor_mul(zn_t[:], z_sb[:], n_sb[:])
    nm_t = gt("nm_t")
    nc.vector.tensor_sub(nm_t[:], n_sb[:], zn_t[:])
    hnew = gt("hnew")
    nc.vector.tensor_add(hnew[:], nm_t[:], zh_t[:])
    nc.sync.dma_start(d["o_h"][:], hnew[:])

    # h_new^T (bf16) -> DRAM for AllGather
    hT_loc = p1.tile([P, KH, BC], BF)
    for k in range(KH):
        pt2 = psum.tile([P, BC], F32, space="PSUM", name="pt_hT", tag="pt")
        nc.tensor.transpose(pt2[:], hnew[:, k * P:(k + 1) * P],
                            ident128[:BC, :BC])
        nc.vector.tensor_copy(hT_loc[:, k, :], pt2[:])
    ag_in = dram.tile([H, BC], BF)
    nc.sync.dma_start(ag_in.rearrange("(k p) b -> p k b", p=P), hT_loc[:])
    return ag_in


def build_nc(debug_taps=False):
    nc = bacc.Bacc("TRN2", target_bir_lowering=False, debug=False,
                   num_devices=N_CORES)

    d = {}
    d["ids"] = nc.dram_tensor("ids", [BC, 1], I32, kind="ExternalInput")
    d["emb"] = nc.dram_tensor("emb", [V, H], BF, kind="ExternalInput")
    d["enc"] = nc.dram_tensor("enc", [L, BC, H], BF, kind="ExternalInput")
    d["encT"] = nc.dram_tensor("encT", [H, BC, L], BF, kind="ExternalInput")
    d["wb"] = nc.dram_tensor("wb", [H], BF, kind="ExternalInput")
    d["wcomb"] = nc.dram_tensor("wcomb", [2 * H, H], BF, kind="ExternalInput")
    d["bcomb"] = nc.dram_tensor("bcomb", [H], F32, kind="ExternalInput")
    d["wih"] = nc.dram_tensor("wih", [H, 3 * H], BF, kind="ExternalInput")
    d["whh"] = nc.dram_tensor("whh", [H, 3 * H], BF, kind="ExternalInput")
    d["bih"] = nc.dram_tensor("bih", [1, 3 * H], BF, kind="ExternalInput")
    d["bhh"] = nc.dram_tensor("bhh", [1, 3 * H], BF, kind="ExternalInput")
    d["h0"] = nc.dram_tensor("h0", [BC, H], F32, kind="ExternalInput")
    d["h0T"] = nc.dram_tensor("h0T", [H, BC], BF, kind="ExternalInput")
    d["wout"] = nc.dram_tensor("wout", [H, VC], BF, kind="ExternalInput")
    d["outb"] = nc.dram_tensor("outb", [1, VC], BF, kind="ExternalInput")
    d["o_logp"] = nc.dram_tensor("out_logp", [B, VC], F32, kind="ExternalOutput")
    if debug_taps:
        d["o_xT"] = nc.dram_tensor("out_xT", [H, BC], F32, kind="ExternalOutput")
        d["o_catT"] = nc.dram_tensor("out_catT", [2 * H, BC], F32,
                                     kind="ExternalOutput")
        d["o_gi"] = nc.dram_tensor("out_gi", [BC, 3 * H], F32,
                                   kind="ExternalOutput")
        d["o_gh"] = nc.dram_tensor("out_gh", [BC, 3 * H], F32,
                                   kind="ExternalOutput")
        d["o_adiag"] = nc.dram_tensor("out_adiag", [BC, L * BC], BF,
                                      kind="ExternalOutput")
        d["o_encrt"] = nc.dram_tensor("out_encrt", [BC, L * H], BF,
                                      kind="ExternalOutput")
    d["o_h"] = nc.dram_tensor("out_h", [BC, H], F32, kind="ExternalOutput")
    d["o_attn"] = nc.dram_tensor("out_attn", [BC, L], F32, kind="ExternalOutput")

    RG = [list(range(N_CORES))]

    with tile.TileContext(nc) as tc:
        with contextlib.ExitStack() as ctx:
            dram = ctx.enter_context(tc.tile_pool(name="dram", bufs=1, space="DRAM"))
            const = ctx.enter_context(tc.tile_pool(name="const", bufs=1))
            persist = ctx.enter_context(tc.tile_pool(name="persist", bufs=1))
            work = ctx.enter_context(tc.tile_pool(name="work", bufs=2))
            wstream = ctx.enter_context(tc.tile_pool(name="wstream", bufs=3))

            ident64 = const.tile([64, 64], BF)
            make_identity(nc, ident64)
            ident128 = const.tile([P, P], F32)
            make_identity(nc, ident128)
            ones_row = const.tile([1, P], BF)
            nc.vector.memset(ones_row[:], 1.0)

            outb_sb = persist.tile([1, VC], BF)
            nc.sync.dma_start(outb_sb[:], d["outb"][:])

            # warm up the collectives firmware with a 1-element AllReduce at
            # t0 so the real AllGather doesn't pay the cold-start; it runs on
            # TOPSP silicon, concurrent with all of phase 1.
            warm_in = dram.tile([1, 128], F32)
            warm_out = dram.tile([1, 128], F32, addr_space="Shared")
            warm_sb = persist.tile([1, 128], F32)
            nc.vector.memset(warm_sb[:], 1.0)
            nc.sync.dma_start(warm_in[:], warm_sb[:])
            nc.gpsimd.collective_compute(
                "AllReduce", mybir.AluOpType.add,
                replica_groups=RG, ins=[warm_in.opt()], outs=[warm_out.opt()],
            )

            # prefetch half the out_w shard (4 x 1MB tiles) during phase 1;
            # the rest streams during phase 2 once phase-1 SBUF is released.
            NPRE = 4
            wout_pool = ctx.enter_context(tc.tile_pool(name="wout_pool", bufs=1))
            wout_tiles = []
            for n in range(NPRE):
                wt = wout_pool.tile([P, KH, VT], BF, name=f"wout_{n}",
                                    tag=f"wout_{n}")
                nc.sync.dma_start(
                    wt[:],
                    d["wout"].ap()[:, n * VT:(n + 1) * VT]
                    .rearrange("(k p) v -> p k v", p=P))
                wout_tiles.append(wt)

            with contextlib.ExitStack() as ctx1:
                ag_in = _phase1(nc, tc, ctx1, d, (ident64, ident128, ones_row),
                                dram, wstream)

            # ---------- AllGather h_new^T across cores ----------
            ag_out = dram.tile([N_CORES * H, BC], BF, addr_space="Shared")
            nc.gpsimd.collective_compute(
                "AllGather", mybir.AluOpType.bypass,
                replica_groups=RG, ins=[ag_in.opt()], outs=[ag_out.opt()],
            )

            p2 = ctx.enter_context(tc.tile_pool(name="p2", bufs=1))
            psum2 = ctx.enter_context(
                tc.tile_pool(name="psum2", bufs=4, space="PSUM"))
            hT_all = p2.tile([P, KH, N_CORES, BC], BF)
            ag_view = ag_out.rearrange("(r k p) b -> p k r b", r=N_CORES, p=P)
            for k in range(KH):
                nc.sync.dma_start(hT_all[:, k, :, :], ag_view[:, k, :, :])

            # ---------- phase 2: logits (DVE evict) + exp sums (ACT) ----------
            logit_sb = p2.tile([P, MB, VC], F32)
            sums = p2.tile([P, MB, NV], F32)
            for n in range(NV):
                if n < NPRE:
                    wt = wout_tiles[n]
                else:
                    wt = wstream.tile([P, KH, VT], BF, name=f"wout_s{n}",
                                      tag="w")
                    nc.sync.dma_start(
                        wt[:],
                        d["wout"].ap()[:, n * VT:(n + 1) * VT]
                        .rearrange("(k p) v -> p k v", p=P))
                for m in range(MB):
                    pl = psum2.tile([P, VT], F32, space="PSUM", name="pl")
                    for k in range(KH):
                        nc.tensor.matmul(
                            pl[:], lhsT=hT_all[:, k, 2 * m:2 * m + 2, :],
                            rhs=wt[:, k, :], start=(k == 0), stop=False)
                    nc.tensor.matmul(pl[:], lhsT=ones_row[:1, :],
                                     rhs=outb_sb[:1, n * VT:(n + 1) * VT],
                                     start=False, stop=True)
                    nc.vector.tensor_copy(
                        logit_sb[:, m, n * VT:(n + 1) * VT], pl[:])
                    esc = work.tile([P, VT], F32, name="esc", tag="esc")
                    nc.scalar.activation(
                        esc[:], pl[:], mybir.ActivationFunctionType.Exp,
                        accum_out=sums[:, m, n:n + 1])

            # ---------- global log-sum-exp via AllReduce ----------
            lsum = p2.tile([P, MB], F32)
            nc.vector.reduce_sum(lsum[:], sums[:], axis=mybir.AxisListType.X)
            ar_in = dram.tile([B, 1], F32)
            nc.sync.dma_start(ar_in.rearrange("(m p) o -> p (m o)", p=P), lsum[:])
            ar_out = dram.tile([B, 1], F32, addr_space="Shared")
            nc.gpsimd.collective_compute(
                "AllReduce", mybir.AluOpType.add,
                replica_groups=RG, ins=[ar_in.opt()], outs=[ar_out.opt()],
            )
            gsum = p2.tile([P, MB], F32)
            nc.sync.dma_start(gsum[:], ar_out.rearrange("(m p) o -> p (m o)", p=P))
            lgs = p2.tile([P, MB], F32)
            nc.scalar.activation(lgs[:], gsum[:],
                                 mybir.ActivationFunctionType.Ln)

            # ---------- final: out = logits - ln(gsum)  (DVE) ----------
            for m in range(MB):
                for n in range(NV):
                    ot = work.tile([P, VT], F32, name="ot", tag="ot")
                    nc.vector.tensor_scalar(
                        out=ot[:], in0=logit_sb[:, m, n * VT:(n + 1) * VT],
                        scalar1=lgs[:, m:m + 1], scalar2=None,
                        op0=mybir.AluOpType.subtract)
                    nc.sync.dma_start(
                        d["o_logp"].ap()[m * P:(m + 1) * P,
                                         n * VT:(n + 1) * VT],
                        ot[:])

    nc.compile()
    return nc


def _prep_in_maps(input_ids, hidden, encoder_outputs, embedding, attn1_w,
                  combine_w, combine_b, w_ih, w_hh, b_ih, b_hh, out_w, out_b):
    emb_b = np.ascontiguousarray(embedding.astype(BF16))
    wcomb = np.ascontiguousarray(combine_w.T.astype(BF16))
    wih = np.ascontiguousarray(w_ih.T.astype(BF16))
    whh = np.ascontiguousarray(w_hh.T.astype(BF16))
    bih = np.ascontiguousarray(b_ih.reshape(1, -1).astype(BF16))
    bhh = np.ascontiguousarray(b_hh.reshape(1, -1).astype(BF16))
    wb = np.ascontiguousarray(attn1_w[0, H:].astype(BF16))
    bcomb = np.ascontiguousarray(combine_b.astype(np.float32))
    in_maps = []
    for c in range(N_CORES):
        sl = slice(c * BC, (c + 1) * BC)
        vsl = slice(c * VC, (c + 1) * VC)
        enc_c = encoder_outputs[:, sl, :]
        h_c = hidden[0, sl, :]
        in_maps.append({
            "ids": np.ascontiguousarray(
                input_ids[sl].reshape(BC, 1).astype(np.int32)),
            "emb": emb_b,
            "enc": np.ascontiguousarray(enc_c.astype(BF16)),
            "encT": np.ascontiguousarray(enc_c.transpose(2, 1, 0).astype(BF16)),
            "wb": wb,
            "wcomb": wcomb,
            "bcomb": bcomb,
            "wih": wih,
            "whh": whh,
            "bih": bih,
            "bhh": bhh,
            "h0": np.ascontiguousarray(h_c.astype(np.float32)),
            "h0T": np.ascontiguousarray(h_c.T.astype(BF16)),
            "wout": np.ascontiguousarray(out_w[vsl, :].T.astype(BF16)),
            "outb": np.ascontiguousarray(out_b[vsl].reshape(1, -1).astype(BF16)),
        })
    return in_maps


def kernel(input_ids, hidden, encoder_outputs, max_length, embedding,
           attn1_w, attn1_b, combine_w, combine_b,
           w_ih, w_hh, b_ih, b_hh, out_w, out_b, _trace=False):
    input_ids = np.asarray(input_ids)
    hidden = np.asarray(hidden, dtype=np.float32)
    encoder_outputs = np.asarray(encoder_outputs, dtype=np.float32)
    embedding = np.asarray(embedding, dtype=np.float32)
    attn1_w = np.asarray(attn1_w, dtype=np.float32)
    combine_w = np.asarray(combine_w, dtype=np.float32)
    combine_b = np.asarray(combine_b, dtype=np.float32)
    w_ih = np.asarray(w_ih, dtype=np.float32)
    w_hh = np.asarray(w_hh, dtype=np.float32)
    b_ih = np.asarray(b_ih, dtype=np.float32)
    b_hh = np.asarray(b_hh, dtype=np.float32)
    out_w = np.asarray(out_w, dtype=np.float32)
    out_b = np.asarray(out_b, dtype=np.float32)
    assert int(max_length) == L

    if "nc" not in _CACHE:
        _CACHE["nc"] = build_nc()
    nc = _CACHE["nc"]

    in_maps = _prep_in_maps(input_ids, hidden, encoder_outputs, embedding,
                            attn1_w, combine_w, combine_b, w_ih, w_hh,
                            b_ih, b_hh, out_w, out_b)
    res = run_bass_kernel_spmd(nc, in_maps, list(range(N_CORES)),
                               trace=_trace)
    if _trace:
        print(f"HW exec time: {res.exec_time_ns} ns")

    output = np.concatenate(
        [res.results[c]["out_logp"] for c in range(N_CORES)], axis=1)
    h_new = np.concatenate(
        [res.results[c]["out_h"] for c in range(N_CORES)], axis=0)[None]
    attn_w = np.concatenate(
        [res.results[c]["out_attn"] for c in range(N_CORES)], axis=0)
    attn_w = np.ascontiguousarray(attn_w.T)[:, :, None]
    return (output.astype(np.float32), h_new.astype(np.float32),
            attn_w.astype(np.float32))
